# revision 1
# baseline (speedup 1.0000x reference)
"""Trainium2 kernel for nn_Conv_RBS_density (496x496 density-matrix RBS circuit).

The reference applies 48 sequential RBS-gate conjugations
``rho <- U rho U^T`` where every ``U = cos(t)*A + sin(t)*B + C`` is an
orthogonal matrix of 30 disjoint 2x2 Givens rotations.  By associativity
the whole pipeline is ``out = V rho V^T`` with ``V = U48 @ ... @ U1``.
V is accumulated on the host with sparse Givens row updates
(O(48*30*N) flops — negligible); the O(N^3) work — the dense matmuls
against rho — runs on the 8 NeuronCores.

Structure: gates only couple basis states within connected components of
the qubit-tile graph, so V is block-diagonal (28 components of 16 states
+ 8 of 6).  Packing components into 4 bins of exactly 124 states gives a
grouped order where V_g = diag(B0, B1, B2, B3), each 124x124.

Sharding: grouped output rows split 8 x 62, no collectives.  Core k
(block B = k//2, half h = k%2) computes, entirely from 124-wide tiles:
  mm1:  P[j]  = rho_g[Brows, jcols].T @ vkt      (4x [124, 62])
        where vkt = (V_g[gk, Bcols]).T; P == (V_k rho_g).T chunks
        because rho is symmetric — this avoids any on-device transpose.
  mm2:  out_k[:, jcols] = P[j].T @ B_j^T         (4x [62, 124])
The host concatenates the 8 row blocks and un-permutes.

DMA layout (measured fastest): per-queue effective DMA bandwidth is only
~58 GB/s and completions serialize per queue, so the 525 KB of per-core
input is spread as 8 small transfers over all three DMA queues
(sync/scalar HWDGE + gpsimd SWDGE), ordered so chunks land just before
the PE needs them; each output chunk is DMA'd as soon as its mm2
finishes.  HW-measured: ~22.4 us end-to-end (of which ~9.5 us is the
fixed Bass kernel preamble/postamble), rel err ~6e-7.  The first
mm1 input ([vkt | rho chunk 0]) rides a single merged sync-first DMA.
"""

import numpy as np

import concourse.mybir as mybir
from concourse import bacc
from concourse.bass import ts
from concourse.bass_utils import run_bass_kernel_spmd
from concourse.tile import TileContext

N = 496          # C(32, 2) Hamming-weight-2 states
NCORES = 8
R = N // NCORES  # 62 output rows per core
BK = 124         # packed block size
NB = N // BK     # 4 blocks

_cache = {}


def _build_program():
    nc = bacc.Bacc(
        "TRN2", target_bir_lowering=False, debug=False, num_devices=NCORES
    )
    f32 = mybir.dt.float32
    # m0 = [vkt | rho block col 0]: one sync-first DMA delivers everything
    # mm1 j0 needs, so the PE starts earliest.  rho carries the full
    # 124-row block (only column chunks 1-3 are DMA'd from it).
    m0_d = nc.dram_tensor("m0", [BK, R + BK], f32, kind="ExternalInput")
    rho_d = nc.dram_tensor("rho", [BK, N], f32, kind="ExternalInput")
    vtb_d = nc.dram_tensor("vtb", [BK, N], f32, kind="ExternalInput")
    out_d = nc.dram_tensor("out", [R, N], f32, kind="ExternalOutput")
    qs = [nc.sync, nc.scalar, nc.gpsimd]

    with TileContext(nc) as tc:
        with (
            tc.tile_pool(name="sbuf", bufs=1) as sbuf,
            tc.tile_pool(name="psum", bufs=1, space="PSUM") as psum,
        ):
            m0_sb = sbuf.tile([BK, R + BK], f32, tag="m0", name="m0")
            nc.sync.dma_start(m0_sb[:], m0_d[:, :])
            vkt_sb = m0_sb[:, 0:R]
            rho_sb = [m0_sb[:, R : R + BK]]
            for j in range(1, NB):
                r = sbuf.tile([BK, BK], f32, tag=f"rho{j}", name=f"rho{j}")
                qs[(j + 1) % 3].dma_start(r[:], rho_d[:, ts(j, BK)])
                rho_sb.append(r)
            vtb_sb = []
            for j in range(NB):
                w = sbuf.tile([BK, BK], f32, tag=f"vtb{j}", name=f"vtb{j}")
                qs[(j + 2) % 3].dma_start(w[:], vtb_d[:, ts(j, BK)])
                vtb_sb.append(w)

            # mm1: P chunks; copy PSUM->SBUF so they can serve as mm2 lhsT
            pk_sb = []
            for j in range(NB):
                pp = psum.tile([BK, R], f32, tag=f"pp{j}", name=f"pp{j}")
                nc.tensor.matmul(
                    pp[:], rho_sb[j][:], vkt_sb[:], start=True, stop=True
                )
                pk = sbuf.tile([BK, R], f32, tag=f"pk{j}", name=f"pk{j}")
                nc.vector.tensor_copy(pk[:], pp[:])
                pk_sb.append(pk)

            # mm2: out column chunks, each DMA'd out as soon as it's ready
            for j in range(NB):
                po = psum.tile([R, BK], f32, tag=f"po{j}", name=f"po{j}")
                nc.tensor.matmul(
                    po[:], pk_sb[j][:], vtb_sb[j][:], start=True, stop=True
                )
                ob = sbuf.tile([R, BK], f32, tag=f"ob{j}", name=f"ob{j}")
                nc.vector.tensor_copy(ob[:], po[:])
                qs[j % 3].dma_start(out_d[:, ts(j, BK)], ob[:])

    nc.compile()
    return nc


def _program():
    if "nc" not in _cache:
        _cache["nc"] = _build_program()
    return _cache["nc"]


def _gate_pairs(B_stack):
    """Per unique gate: (s, q) index arrays with B[u, s, q] = +1."""
    pairs = []
    for u in range(B_stack.shape[0]):
        pos = np.argwhere(B_stack[u] > 0.5)
        pairs.append((pos[:, 0], pos[:, 1]))
    return pairs


def _build_V(thetas, pairs, u_idx, p_idx, n):
    """V = U_G ... U_1 via sparse Givens row updates (float64)."""
    thetas = np.asarray(thetas, np.float64)
    cos_t, sin_t = np.cos(thetas), np.sin(thetas)
    V = np.eye(n)
    for g in range(len(u_idx)):
        u, p = int(u_idx[g]), int(p_idx[g])
        c, s = cos_t[p], sin_t[p]
        S, Q = pairs[u]
        vs, vq = V[S], V[Q]
        V[S] = c * vs + s * vq
        V[Q] = -s * vs + c * vq
    return V


def _grouping(pairs, n):
    """Union states coupled by any gate; pack components into NB bins of BK."""
    parent = list(range(n))

    def find(a):
        while parent[a] != a:
            parent[a] = parent[parent[a]]
            a = parent[a]
        return a

    for S, Q in pairs:
        for s, q in zip(S.tolist(), Q.tolist()):
            ra, rb = find(s), find(q)
            if ra != rb:
                parent[ra] = rb

    comps = {}
    for i in range(n):
        comps.setdefault(find(i), []).append(i)
    comps = sorted(comps.values(), key=len, reverse=True)

    bins = [[] for _ in range(NB)]
    for comp in comps:
        for b in bins:
            if len(b) + len(comp) <= BK:
                b.extend(comp)
                break
        else:
            raise ValueError("component packing failed")
    assert all(len(b) == BK for b in bins), [len(b) for b in bins]
    return np.array([i for b in bins for i in b], np.int64)


def _run(rho, thetas, A_stack, B_stack, C_stack, u_idx, p_idx, trace=False):
    rho = np.asarray(rho, np.float32)
    B_stack = np.asarray(B_stack)
    u_idx = np.asarray(u_idx).astype(np.int64)
    p_idx = np.asarray(p_idx).astype(np.int64)
    n = rho.shape[0]
    assert n == N, n

    if "struct" not in _cache:
        pairs = _gate_pairs(B_stack)
        _cache["struct"] = (pairs, _grouping(pairs, n))
    pairs, perm = _cache["struct"]

    V = _build_V(thetas, pairs, u_idx, p_idx, n).astype(np.float32)
    V_g = V[np.ix_(perm, perm)]
    rho_g = np.ascontiguousarray(rho[np.ix_(perm, perm)])

    # block-diagonality check (structure is fixed by the module definition)
    blocks = [
        V_g[j * BK : (j + 1) * BK, j * BK : (j + 1) * BK] for j in range(NB)
    ]
    bd = np.zeros_like(V_g)
    for j in range(NB):
        bd[j * BK : (j + 1) * BK, j * BK : (j + 1) * BK] = blocks[j]
    assert np.array_equal(bd, V_g), "V lost block-diagonal structure"

    # vtb: [124, 496], column block j = B_j^T (shared by all cores)
    vtb = np.ascontiguousarray(np.concatenate([b.T for b in blocks], axis=1))

    in_maps = []
    for k in range(NCORES):
        B, h = divmod(k, 2)
        rho_rows = np.ascontiguousarray(rho_g[B * BK : (B + 1) * BK, :])
        m0 = np.concatenate(
            [blocks[B][h * R : (h + 1) * R, :].T, rho_rows[:, 0:BK]], axis=1
        )
        in_maps.append(
            {
                "m0": np.ascontiguousarray(m0),
                "rho": rho_rows,
                "vtb": vtb,
            }
        )

    res = run_bass_kernel_spmd(_program(), in_maps, list(range(NCORES)), trace=trace)
    out_g = np.concatenate(
        [res.results[k]["out"] for k in range(NCORES)], axis=0
    )
    out = np.empty((n, n), np.float32)
    out[np.ix_(perm, perm)] = out_g
    return out, res


def kernel(rho, thetas, A_stack, B_stack, C_stack, u_idx, p_idx):
    out, _ = _run(rho, thetas, A_stack, B_stack, C_stack, u_idx, p_idx)
    return out



# revision 2
# speedup vs baseline: 1.2696x; 1.2696x over previous
"""Trainium2 kernel for nn_Conv_RBS_density (496x496 density-matrix RBS circuit).

The reference applies 48 sequential RBS-gate conjugations
``rho <- U rho U^T`` where every ``U = cos(t)*A + sin(t)*B + C`` is an
orthogonal matrix of 30 disjoint 2x2 Givens rotations.  By associativity
the whole pipeline is ``out = V rho V^T`` with ``V = U48 @ ... @ U1``.
V is accumulated on the host with sparse Givens row updates
(O(48*30*N) flops - negligible); the O(N^3) work - the dense matmuls
against rho - runs on the 8 NeuronCores.

Structure: gates only couple basis states within connected components of
the qubit-tile graph, so V is block-diagonal (28 components of 16 states
+ 8 of 6).  Packing components into 4 bins of exactly 124 states gives a
grouped order where V_g = diag(B0, B1, B2, B3), each 124x124, and
``out_g[i,j] = B_i rho_ij B_j^T`` for the 16 (i,j) 124x124 blocks.

Sharding: 2 output blocks per core (core c: i = c//2, j in {2*(c%2),
2*(c%2)+1}), no collectives.  Per block, using rho_ij = rho_ji^T (rho
symmetric) to avoid on-device transposes:
  mm1:  P  = matmul(lhsT=rho_ij, rhs=B_i^T) = (B_i rho_ij)^T
  mm2:  O  = matmul(lhsT=P,      rhs=B_j^T) = B_i rho_ij B_j^T
All inputs are bf16 (PE runs single-pass instead of the fp32 LOW/HIGH
double pass; DMA bytes halve); accumulation is fp32 in PSUM and the
output is copied out as fp32.  Max rel err ~4e-3, well under the 2e-2
gate.  Inputs ride 3 DMAs (sync/scalar/gpsimd queues) with 512B-aligned
partition lines; each output block is DMA'd the moment its copy lands.
"""

import numpy as np
import ml_dtypes

import concourse.mybir as mybir
from concourse import bacc
from concourse.bass_utils import run_bass_kernel_spmd
from concourse.tile import TileContext

N = 496          # C(32, 2) Hamming-weight-2 states
NCORES = 8
BK = 124         # packed block size
NB = N // BK     # 4 blocks
PAD = 256        # input cols padded so DMA partition lines are 512B

BF16 = ml_dtypes.bfloat16

_cache = {}


def _build_program():
    nc = bacc.Bacc(
        "TRN2", target_bir_lowering=False, debug=False, num_devices=NCORES
    )
    bf = mybir.dt.bfloat16
    f32 = mybir.dt.float32
    # xa = [rho_ij0 | B_i^T | pad], xb = [B_j0^T | rho_ij1 | pad],
    # xc = [B_j1^T | pad]; all [124, 256] bf16.
    xa_d = nc.dram_tensor("xa", [BK, PAD], bf, kind="ExternalInput")
    xb_d = nc.dram_tensor("xb", [BK, PAD], bf, kind="ExternalInput")
    xc_d = nc.dram_tensor("xc", [BK, PAD], bf, kind="ExternalInput")
    o0_d = nc.dram_tensor("o0", [BK, BK], f32, kind="ExternalOutput")
    o1_d = nc.dram_tensor("o1", [BK, BK], f32, kind="ExternalOutput")

    with TileContext(nc) as tc:
        with (
            tc.tile_pool(name="sbuf", bufs=1) as sbuf,
            tc.tile_pool(name="psum", bufs=1, space="PSUM") as psum,
        ):
            xa = sbuf.tile([BK, PAD], bf, tag="xa", name="xa")
            nc.sync.dma_start(xa[:], xa_d[:, :])
            xb = sbuf.tile([BK, PAD], bf, tag="xb", name="xb")
            nc.scalar.dma_start(xb[:], xb_d[:, :])
            xc = sbuf.tile([BK, PAD], bf, tag="xc", name="xc")
            nc.gpsimd.dma_start(xc[:], xc_d[:, :])

            r0, bi = xa[:, 0:BK], xa[:, BK : 2 * BK]
            bj0, r1 = xb[:, 0:BK], xb[:, BK : 2 * BK]
            bj1 = xc[:, 0:BK]

            # mm1 for both blocks back-to-back so the PE never waits on
            # the PSUM->SBUF copies.
            p0 = psum.tile([BK, BK], f32, tag="p0", name="p0")
            nc.tensor.matmul(p0[:], r0, bi, start=True, stop=True)
            p1 = psum.tile([BK, BK], f32, tag="p1", name="p1")
            nc.tensor.matmul(p1[:], r1, bi, start=True, stop=True)

            pk0 = sbuf.tile([BK, BK], bf, tag="pk0", name="pk0")
            nc.vector.tensor_copy(pk0[:], p0[:])
            pk1 = sbuf.tile([BK, BK], bf, tag="pk1", name="pk1")
            nc.scalar.copy(pk1[:], p1[:])

            o0 = psum.tile([BK, BK], f32, tag="o0", name="o0")
            nc.tensor.matmul(o0[:], pk0[:], bj0, start=True, stop=True)
            o1 = psum.tile([BK, BK], f32, tag="o1", name="o1")
            nc.tensor.matmul(o1[:], pk1[:], bj1, start=True, stop=True)

            ob0 = sbuf.tile([BK, BK], f32, tag="ob0", name="ob0")
            nc.vector.tensor_copy(ob0[:], o0[:])
            nc.sync.dma_start(o0_d[:, :], ob0[:])
            ob1 = sbuf.tile([BK, BK], f32, tag="ob1", name="ob1")
            nc.scalar.copy(ob1[:], o1[:])
            nc.scalar.dma_start(o1_d[:, :], ob1[:])

    nc.compile()
    return nc


def _program():
    if "nc" not in _cache:
        _cache["nc"] = _build_program()
    return _cache["nc"]


def _gate_pairs(B_stack):
    """Per unique gate: (s, q) index arrays with B[u, s, q] = +1."""
    pairs = []
    for u in range(B_stack.shape[0]):
        pos = np.argwhere(B_stack[u] > 0.5)
        pairs.append((pos[:, 0], pos[:, 1]))
    return pairs


def _build_V(thetas, pairs, u_idx, p_idx, n):
    """V = U_G ... U_1 via sparse Givens row updates (float64)."""
    thetas = np.asarray(thetas, np.float64)
    cos_t, sin_t = np.cos(thetas), np.sin(thetas)
    V = np.eye(n)
    for g in range(len(u_idx)):
        u, p = int(u_idx[g]), int(p_idx[g])
        c, s = cos_t[p], sin_t[p]
        S, Q = pairs[u]
        vs, vq = V[S], V[Q]
        V[S] = c * vs + s * vq
        V[Q] = -s * vs + c * vq
    return V


def _grouping(pairs, n):
    """Union states coupled by any gate; pack components into NB bins of BK."""
    parent = list(range(n))

    def find(a):
        while parent[a] != a:
            parent[a] = parent[parent[a]]
            a = parent[a]
        return a

    for S, Q in pairs:
        for s, q in zip(S.tolist(), Q.tolist()):
            ra, rb = find(s), find(q)
            if ra != rb:
                parent[ra] = rb

    comps = {}
    for i in range(n):
        comps.setdefault(find(i), []).append(i)
    comps = sorted(comps.values(), key=len, reverse=True)

    bins = [[] for _ in range(NB)]
    for comp in comps:
        for b in bins:
            if len(b) + len(comp) <= BK:
                b.extend(comp)
                break
        else:
            raise ValueError("component packing failed")
    assert all(len(b) == BK for b in bins), [len(b) for b in bins]
    return np.array([i for b in bins for i in b], np.int64)


def _run(rho, thetas, A_stack, B_stack, C_stack, u_idx, p_idx, trace=False):
    rho = np.asarray(rho, np.float32)
    B_stack = np.asarray(B_stack)
    u_idx = np.asarray(u_idx).astype(np.int64)
    p_idx = np.asarray(p_idx).astype(np.int64)
    n = rho.shape[0]
    assert n == N, n

    if "struct" not in _cache:
        pairs = _gate_pairs(B_stack)
        _cache["struct"] = (pairs, _grouping(pairs, n))
    pairs, perm = _cache["struct"]

    V = _build_V(thetas, pairs, u_idx, p_idx, n).astype(np.float32)
    V_g = V[np.ix_(perm, perm)]
    rho_g = np.ascontiguousarray(rho[np.ix_(perm, perm)])

    # block-diagonality check (structure is fixed by the module definition)
    blocks = [
        V_g[j * BK : (j + 1) * BK, j * BK : (j + 1) * BK] for j in range(NB)
    ]
    bd = np.zeros_like(V_g)
    for j in range(NB):
        bd[j * BK : (j + 1) * BK, j * BK : (j + 1) * BK] = blocks[j]
    assert np.array_equal(bd, V_g), "V lost block-diagonal structure"

    rho_bf = rho_g.astype(BF16)
    bT = [np.ascontiguousarray(b.T).astype(BF16) for b in blocks]
    zpad = np.zeros((BK, PAD - 2 * BK), BF16)
    zpadc = np.zeros((BK, PAD - BK), BF16)

    in_maps = []
    for c in range(NCORES):
        i, pr = divmod(c, 2)
        j0, j1 = 2 * pr, 2 * pr + 1
        R0 = rho_bf[i * BK : (i + 1) * BK, j0 * BK : (j0 + 1) * BK]
        R1 = rho_bf[i * BK : (i + 1) * BK, j1 * BK : (j1 + 1) * BK]
        in_maps.append(
            {
                "xa": np.ascontiguousarray(
                    np.concatenate([R0, bT[i], zpad], axis=1)
                ),
                "xb": np.ascontiguousarray(
                    np.concatenate([bT[j0], R1, zpad], axis=1)
                ),
                "xc": np.ascontiguousarray(
                    np.concatenate([bT[j1], zpadc], axis=1)
                ),
            }
        )

    res = run_bass_kernel_spmd(
        _program(), in_maps, list(range(NCORES)), trace=trace
    )
    out_g = np.empty((n, n), np.float32)
    for c in range(NCORES):
        i, pr = divmod(c, 2)
        j0, j1 = 2 * pr, 2 * pr + 1
        out_g[i * BK : (i + 1) * BK, j0 * BK : (j0 + 1) * BK] = np.asarray(
            res.results[c]["o0"], np.float32
        )
        out_g[i * BK : (i + 1) * BK, j1 * BK : (j1 + 1) * BK] = np.asarray(
            res.results[c]["o1"], np.float32
        )
    out = np.empty((n, n), np.float32)
    out[np.ix_(perm, perm)] = out_g
    return out, res


def kernel(rho, thetas, A_stack, B_stack, C_stack, u_idx, p_idx):
    out, _ = _run(rho, thetas, A_stack, B_stack, C_stack, u_idx, p_idx)
    return out
